# revision 1
# baseline (speedup 1.0000x reference)
"""Trainium2 Bass kernel for nn_Attention_85813446574600.

Reference computes:
    s_x = x @ W[:F] + b            # [B,T,1]
    s_c = context @ W[F:]          # [C,1]
    scores = s_x + s_c             # [B,T,C,1]
    att = softmax(scores, axis=-1) # softmax over a SIZE-1 axis -> exactly 1.0
    out = einsum('btc,btf->bcf', att, x)

Since softmax over the last (size-1) axis is identically 1.0 for any finite
scores, the output is exactly out[b,c,f] = sum_t x[b,t,f], independent of c
(and of context/W/b entirely).

Device kernel (per core, batch-sharded 32/8 = 4 batches), raw Bass (no Tile
framework -- avoids its entry/exit barrier overhead):

  sync engine   : all input DMAs on the qSP HWDGE ring (two concurrent
                  rings were measured SLOWER: 2x146GB/s vs 1x323GB/s).
                  Partition p holds consecutive T rows, giving
                  per-partition-contiguous 8KB descriptors. The last batch is
                  split into two half-loads so its reduction starts while the
                  second half is still streaming.
  sync+scalar   : each batch's [256,512] output slab is written as two
                  128-row halves, one per HWDGE ring, so the final batch is
                  never queued behind an earlier transfer.
  vector engine : pre-reduce the T rows in each partition with wide adds,
                  then copy the matmul result PSUM->SBUF. copy(1) is placed
                  in the DVE idle window while batch 3's second half streams.
  tensor engine : ONES[128,128] @ total -> PSUM; an all-ones stationary
                  matrix both sums across partitions and broadcasts the
                  result to all 128 output partitions in one matmul. Dummy
                  warm-up/filler matmuls keep the PE HAM throttle at the
                  warm clock for the latency-critical real matmuls.
"""

import sys

for _p in ("/opt/trn_rl_repo",):
    if _p not in sys.path:
        sys.path.insert(0, _p)

from contextlib import ExitStack

import numpy as np

import concourse.bass as bass
import concourse.mybir as mybir
from concourse.bass_utils import run_bass_kernel_spmd

# Problem shapes (hardcoded per harness contract)
B, T, C, F = 32, 512, 256, 512
N_CORES = 8
B_LOC = B // N_CORES  # 4 batches per core
P = 128               # SBUF/PSUM partitions
TT = T // P           # 4 T-rows folded into each partition
DT = mybir.dt.float32

_NC_CACHE = {}


def _build_nc():
    # Bass.__init__ ends with const-AP memsets plus an all-engine barrier;
    # nothing in this kernel reads the const APs and every cross-engine
    # dependency is explicitly semaphore-gated, so skip that barrier to
    # issue the first input DMA ~0.4us sooner.
    _orig_barrier = bass.Bass.all_engine_barrier
    bass.Bass.all_engine_barrier = lambda self, sem_only=False: None
    try:
        nc = bass.Bass("TRN2", target_bir_lowering=False)
    finally:
        bass.Bass.all_engine_barrier = _orig_barrier
    x = nc.dram_tensor("x", [B_LOC, T, F], DT, kind="ExternalInput").ap()
    out = nc.dram_tensor("out", [B_LOC, C, F], DT, kind="ExternalOutput").ap()

    with ExitStack() as ctx:
        ec = ctx.enter_context
        ones = ec(nc.sbuf_tensor("ones", [P, P], DT)).ap()
        # b0..b2: one [128, 4*F] tile each; b3: two [128, 2*F] half tiles
        xts = [
            ec(nc.sbuf_tensor(f"xt{b}", [P, TT * F], DT)).ap() for b in range(3)
        ]
        xt3a = ec(nc.sbuf_tensor("xt3a", [P, 2 * F], DT)).ap()
        xt3b = ec(nc.sbuf_tensor("xt3b", [P, 2 * F], DT)).ap()
        pairs = [
            ec(nc.sbuf_tensor(f"pair{b}", [P, 2 * F], DT)).ap() for b in range(3)
        ]
        t3a = ec(nc.sbuf_tensor("t3a", [P, F], DT)).ap()
        t3b = ec(nc.sbuf_tensor("t3b", [P, F], DT)).ap()
        totals = [
            ec(nc.sbuf_tensor(f"total{b}", [P, F], DT)).ap() for b in range(B_LOC)
        ]
        ots = [ec(nc.sbuf_tensor(f"ot{b}", [P, F], DT)).ap() for b in range(B_LOC)]
        accs = [ec(nc.psum_tensor(f"acc{b}", [P, F], DT)).ap() for b in range(3)]
        acc3L = ec(nc.psum_tensor("acc3L", [P, F // 2], DT)).ap()
        acc3R = ec(nc.psum_tensor("acc3R", [P, F // 2], DT)).ap()
        warm_ps = ec(nc.psum_tensor("warm_ps", [P, P], DT)).ap()

        in_sems = [ec(nc.semaphore(f"in_sem{b}")) for b in range(3)]
        in3a_sem = ec(nc.semaphore("in3a_sem"))
        in3b_sem = ec(nc.semaphore("in3b_sem"))
        vec_sem = ec(nc.semaphore("vec_sem"))
        vv_sem = ec(nc.semaphore("vv_sem"))
        pe_sem = ec(nc.semaphore("pe_sem"))
        cp_sem = ec(nc.semaphore("cp_sem"))
        osem_sp = ec(nc.semaphore("osem_sp"))
        osem_act = ec(nc.semaphore("osem_act"))
        v3L_sem = ec(nc.semaphore("v3L_sem"))
        v3R_sem = ec(nc.semaphore("v3R_sem"))
        pe3L_sem = ec(nc.semaphore("pe3L_sem"))
        pe3R_sem = ec(nc.semaphore("pe3R_sem"))
        cp3L_sem = ec(nc.semaphore("cp3L_sem"))
        cp3R_sem = ec(nc.semaphore("cp3R_sem"))

        block = ec(nc.Block())

        def in_dma(eng, b):
            # partition p <- x[b, TT*p : TT*(p+1), :], contiguous 8KB/partition
            src = x[b].rearrange("(p l) f -> p l f", p=P)
            return eng.dma_start(
                xts[b].rearrange("p (l f) -> p l f", l=TT), src
            ).then_inc(in_sems[b], 16)

        def out_half(eng, b, h, sem):
            # one 128-row half of out[b]
            dst = out[b, h * P : (h + 1) * P, :]
            return eng.dma_start(dst, ots[b]).then_inc(sem, 16)

        @block.sync
        def _(sync):
            in_dma(sync, 0)
            in_dma(sync, 1)
            in_dma(sync, 2)
            src3 = x[3].rearrange("(h p l) f -> h p l f", h=2, p=P)
            sync.dma_start(
                xt3a.rearrange("p (l f) -> p l f", l=2), src3[0]
            ).then_inc(in3a_sem, 16)
            sync.dma_start(
                xt3b.rearrange("p (l f) -> p l f", l=2), src3[1]
            ).then_inc(in3b_sem, 16)
            # every output slab is split half/half across the two HWDGE
            # rings so the last batch is never queued behind an earlier one;
            # batch 3 additionally splits into column halves so its left
            # half streams while the right half is still in the matmul
            Fh = F // 2
            for b in range(3):
                sync.wait_ge(cp_sem, b + 1)
                out_half(sync, b, 0, osem_sp)
            sync.wait_ge(cp3L_sem, 1)
            sync.dma_start(out[3, 0:P, 0:Fh], ots[3][:, 0:Fh]).then_inc(osem_sp, 16)
            sync.wait_ge(cp3R_sem, 1)
            sync.dma_start(out[3, 0:P, Fh:F], ots[3][:, Fh:F]).then_inc(osem_sp, 16)
            sync.wait_ge(osem_sp, 16 * 5)

        @block.scalar
        def _(scalar):
            Fh = F // 2
            for b in range(3):
                scalar.wait_ge(cp_sem, b + 1)
                out_half(scalar, b, 1, osem_act)
            scalar.wait_ge(cp3L_sem, 1)
            scalar.dma_start(out[3, P:C, 0:Fh], ots[3][:, 0:Fh]).then_inc(
                osem_act, 16
            )
            scalar.wait_ge(cp3R_sem, 1)
            scalar.dma_start(out[3, P:C, Fh:F], ots[3][:, Fh:F]).then_inc(
                osem_act, 16
            )
            scalar.wait_ge(osem_act, 16 * 5)

        @block.vector
        def _(vector):
            nc.vector.memset(ones, 1.0).then_inc(vec_sem, 1)

            def adds(b):
                vector.wait_ge(in_sems[b], 16)
                nc.vector.tensor_add(
                    pairs[b], xts[b][:, 0 : 2 * F], xts[b][:, 2 * F : 4 * F]
                ).then_inc(vv_sem, 1)
                # same-engine RAW: the DVE pipeline is deep, so the dependent
                # read must wait on the writer's semaphore
                vector.wait_ge(vv_sem, b + 1)
                nc.vector.tensor_add(
                    totals[b], pairs[b][:, 0:F], pairs[b][:, F : 2 * F]
                ).then_inc(vec_sem, 1)

            def copy(b):
                vector.wait_ge(pe_sem, b + 1)
                nc.vector.tensor_copy(ots[b], accs[b]).then_inc(cp_sem, 1)

            adds(0)
            adds(1)
            copy(0)
            adds(2)
            # batch-3 reductions interleaved with the copies: copy(1) fits
            # in the DVE idle gap while b3's second half streams. The final
            # adds/copies run at half-F granularity so the left column half
            # reaches the output ring while the right half still computes.
            Fh = F // 2
            vector.wait_ge(in3a_sem, 16)
            nc.vector.tensor_add(t3a, xt3a[:, 0:F], xt3a[:, F : 2 * F]).then_inc(
                vv_sem, 1
            )
            copy(1)
            vector.wait_ge(in3b_sem, 16)
            nc.vector.tensor_add(
                t3b[:, 0:Fh], xt3b[:, 0:Fh], xt3b[:, F : F + Fh]
            ).then_inc(vv_sem, 1)
            nc.vector.tensor_add(
                t3b[:, Fh:F], xt3b[:, Fh:F], xt3b[:, F + Fh : 2 * F]
            ).then_inc(vv_sem, 1)
            vector.wait_ge(vv_sem, 5)
            nc.vector.tensor_add(
                totals[3][:, 0:Fh], t3a[:, 0:Fh], t3b[:, 0:Fh]
            ).then_inc(v3L_sem, 1)
            vector.wait_ge(vv_sem, 6)
            nc.vector.tensor_add(
                totals[3][:, Fh:F], t3a[:, Fh:F], t3b[:, Fh:F]
            ).then_inc(v3R_sem, 1)
            copy(2)
            vector.wait_ge(pe3L_sem, 1)
            nc.vector.tensor_copy(ots[3][:, 0:Fh], acc3L).then_inc(cp3L_sem, 1)
            vector.wait_ge(pe3R_sem, 1)
            nc.vector.tensor_copy(ots[3][:, Fh:F], acc3R).then_inc(cp3R_sem, 1)

        @block.tensor
        def _(tensor):
            # HAM warm-up: ~7us of dummy matmuls during the input stream so
            # the PE clock is throttled up before the latency-critical real
            # matmuls (cold 1.2GHz vs warm 2.4GHz)
            tensor.wait_ge(vec_sem, 1)
            for _ in range(22):
                nc.tensor.matmul(warm_ps, ones, ones, start=True, stop=True)
            # fillers between the real matmuls keep the HAM window busy so
            # every latency-critical matmul runs at the warm clock
            Fh = F // 2
            fillers = [0, 5, 6]
            for b in range(3):
                for _ in range(fillers[b]):
                    nc.tensor.matmul(warm_ps, ones, ones, start=True, stop=True)
                tensor.wait_ge(vec_sem, b + 2)
                nc.tensor.matmul(
                    accs[b], ones, totals[b], start=True, stop=True
                ).then_inc(pe_sem, 1)
            for _ in range(2):
                nc.tensor.matmul(warm_ps, ones, ones, start=True, stop=True)
            tensor.wait_ge(v3L_sem, 1)
            nc.tensor.matmul(
                acc3L, ones, totals[3][:, 0:Fh], start=True, stop=True
            ).then_inc(pe3L_sem, 1)
            tensor.wait_ge(v3R_sem, 1)
            nc.tensor.matmul(
                acc3R, ones, totals[3][:, Fh:F], start=True, stop=True
            ).then_inc(pe3R_sem, 1)

    return nc


def _get_nc():
    if "nc" not in _NC_CACHE:
        _NC_CACHE["nc"] = _build_nc()
    return _NC_CACHE["nc"]


def kernel(x, context=None, W=None, b=None, **_unused):
    """Full inputs in, full output out. context/W/b provably do not affect
    the output (softmax over a size-1 axis is identically 1)."""
    x = np.ascontiguousarray(np.asarray(x), dtype=np.float32)
    assert x.shape == (B, T, F), x.shape

    nc = _get_nc()
    in_maps = [{"x": x[i * B_LOC : (i + 1) * B_LOC]} for i in range(N_CORES)]
    res = run_bass_kernel_spmd(nc, in_maps, core_ids=list(range(N_CORES)))
    return np.concatenate([r["out"] for r in res.results], axis=0)

